# revision 1
# baseline (speedup 1.0000x reference)
"""GQA dense-transformer block (RMSNorm + QKV + RoPE + causal GQA attention
+ o_proj + residual) on 8 trn2 NeuronCores.

Sharding: 2 (batch) x 4 (head-group tensor parallel). Core c = 4*b + g handles
batch b, q-heads 8g..8g+7, kv-heads 2g..2g+1. Each core produces a partial
o_proj output (+ the RMS-normed residual on g==0 cores only, via a zeroed
rms broadcast on g!=0); the host sums the 4 partials per batch (the TP
"all-reduce" done at unshard time).

On-device layout: "feature-major" attention. Projections compute Q^T/K^T
([head_dim, tokens]) so that scores are produced TRANSPOSED ([k, q]) and the
softmax needs no probs transpose: exp has no max-subtraction (scores are
O(5), safe in fp32), the denominator comes from a ones-column appended to V
(PV matmul M=65), and the division is deferred to a per-head broadcast
multiply after PV.
"""

import math
import numpy as np

# model dims (hardcoded per contract)
B, S, D = 2, 2048, 2048
HQ, HKV, HD = 32, 8, 64
NC = 8
NG = 4            # head groups
QH = 8            # q heads per core
KH = 2            # kv heads per core
CQ = QH * HD      # 512 q cols per core
W512 = S // 512   # 4 token windows
NT = S // 128     # 16 token tiles
NDC = D // 128    # 16 contraction chunks
PERM = [0, 4, 1, 5, 2, 6, 3, 7]  # local head order: ptile p = (h=p | h=p+4)

_cache = {}
_patched = [False]


def _legalize_bir_bytes(bir):
    """Walrus in this container accepts at most ONE embedded sem-wait per TPB
    instruction ("Too many sync wait commands"). Tile emits several when an
    instruction depends on multiple DMA queues. Split the extras into
    standalone EventSemaphore (pure-wait) instructions on the same engine
    immediately before the instruction — identical blocking semantics."""
    import json
    d = json.loads(bir if isinstance(bir, str) else bir.decode())
    n_split = 0
    stack = [d]
    while stack:
        o = stack.pop()
        if isinstance(o, dict):
            insts = o.get("instructions")
            if isinstance(insts, list) and insts and isinstance(insts[0], dict) \
               and "opcode" in insts[0]:
                new = []
                for inst in insts:
                    si = inst.get("sync_info") or {}
                    ws = si.get("on_wait") or []
                    if len(ws) > 1 and isinstance(inst.get("opcode"), str) \
                       and inst.get("opcode") not in (
                            "EventSemaphore", "UnconditionalBranch",
                            "Call", "ISA"):
                        for k, w in enumerate(ws[:-1]):
                            n_split += 1
                            new.append({
                                "debug": inst.get("debug", 0),
                                "engine": inst["engine"],
                                "ins": [], "outs": [],
                                "name": f"lw{n_split}_{inst['name']}",
                                "opcode": "EventSemaphore",
                                "sync_info": {"on_update": [], "on_wait": [w]},
                            })
                        si["on_wait"] = [ws[-1]]
                    new.append(inst)
                o["instructions"] = new
            else:
                stack.extend(o.values())
        elif isinstance(o, list):
            stack.extend(o)
    return json.dumps(d).encode()


def _install_patch():
    if _patched[0]:
        return
    from concourse import bass_utils as bu
    from concourse import bass2jax as b2j
    orig = bu.compile_bir_kernel

    def patched(bir, *a, **k):
        return orig(_legalize_bir_bytes(bir), *a, **k)

    bu.compile_bir_kernel = patched
    b2j.compile_bir_kernel = patched
    _patched[0] = True


def _build(causal: bool):
    import concourse.bass as bass
    import concourse.mybir as mybir
    from concourse.tile import TileContext

    fp32 = mybir.dt.float32
    AF = mybir.ActivationFunctionType

    nc = bass.Bass("TRN2")
    xT = nc.dram_tensor("xT", [D, S], fp32, kind="ExternalInput")
    x = nc.dram_tensor("x", [S, D], fp32, kind="ExternalInput")
    wq = nc.dram_tensor("wq", [D, CQ], fp32, kind="ExternalInput")
    wk = nc.dram_tensor("wk", [D, KH * HD], fp32, kind="ExternalInput")
    wv = nc.dram_tensor("wv", [D, KH * HD], fp32, kind="ExternalInput")
    wo = nc.dram_tensor("wo", [CQ, D], fp32, kind="ExternalInput")
    cosT_d = nc.dram_tensor("cosT", [128, S], fp32, kind="ExternalInput")
    sinT_d = nc.dram_tensor("sinT", [128, S], fp32, kind="ExternalInput")
    maskb_d = nc.dram_tensor("maskb", [128, 896], fp32, kind="ExternalInput")
    rmsf_d = nc.dram_tensor("rmsf", [128, D], fp32, kind="ExternalInput")
    out = nc.dram_tensor("out", [S, D], fp32, kind="ExternalOutput")

    with TileContext(nc) as tc:
        with (
            tc.tile_pool(name="res", bufs=1) as res,
            tc.tile_pool(name="dram", bufs=1, space="DRAM") as dpool,
        ):
            # resident tiles
            QT = [res.tile([128, S], fp32, tag=f"qt{p}", name=f"qt{p}") for p in range(4)]
            KT = res.tile([128, S], fp32, tag="kt", name="kt")
            AT = [res.tile([128, S], fp32, tag=f"at{p}", name=f"at{p}") for p in range(4)]
            v_all = res.tile([128, NT * 130], fp32, tag="vall", name="vall")
            cosT = res.tile([128, S], fp32, tag="cosT")
            sinT = res.tile([128, S], fp32, tag="sinT")
            sbc = res.tile([128, S], fp32, tag="sbc", name="sbc")
            maskb = res.tile([128, 896], fp32, tag="maskb", name="maskb")
            rmsf = res.tile([128, D], fp32, tag="rmsf", name="rmsf")
            s_all = res.tile([128, NT], fp32, tag="sall", name="sall")
            ones1 = res.tile([1, 128], fp32, tag="ones1", name="ones1")
            epst = res.tile([128, 1], fp32, tag="epst", name="epst")
            s_free = res.tile([1, S], fp32, tag="sfree", name="sfree")
            s_dram = dpool.tile([S, 1], fp32, tag="sdram", name="sdram")

            nc.vector.memset(ones1[:, :], 1.0)
            nc.vector.memset(epst[:, :], float(np.finfo(np.float32).eps))
            for tt in range(NT):
                nc.vector.memset(v_all[:, 130 * tt + 64 : 130 * tt + 65], 1.0)
                nc.vector.memset(v_all[:, 130 * tt + 129 : 130 * tt + 130], 1.0)
            nc.gpsimd.dma_start(out=cosT[:, :], in_=cosT_d[:, :])
            nc.gpsimd.dma_start(out=sinT[:, :], in_=sinT_d[:, :])
            nc.gpsimd.dma_start(out=maskb[:, :], in_=maskb_d[:, :])
            nc.gpsimd.dma_start(out=rmsf[:, :], in_=rmsf_d[:, :])

            # ---- phase S: per-token rsqrt(mean(x^2)+eps) ----
            with tc.tile_pool(name="stat", bufs=3) as sp:
                for tt in range(NT):
                    x_t = sp.tile([128, D], fp32, tag="xs", name="xs")
                    nc.gpsimd.dma_start(out=x_t[:, :], in_=x[tt * 128 : (tt + 1) * 128, :])
                    st = sp.tile([128, 4, 6], fp32, tag="st", name="st")
                    for c in range(4):
                        nc.vector.bn_stats(out=st[:, c, :], in_=x_t[:, 512 * c : 512 * (c + 1)])
                    mv = sp.tile([128, 2], fp32, tag="mv", name="mv")
                    nc.vector.bn_aggr(out=mv[:, :], in_=st[:, :, :])
                    msq = sp.tile([128, 1], fp32, tag="msq", name="msq")
                    nc.vector.tensor_mul(msq[:, :], mv[:, 0:1], mv[:, 0:1])
                    nc.vector.tensor_add(msq[:, :], msq[:, :], mv[:, 1:2])
                    sq = sp.tile([128, 1], fp32, tag="sq", name="sq")
                    nc.scalar.activation(out=sq[:, :], in_=msq[:, :], func=AF.Sqrt,
                                         bias=epst[:, 0:1], scale=1.0)
                    nc.vector.reciprocal(out=s_all[:, tt : tt + 1], in_=sq[:, :])
            nc.gpsimd.dma_start(
                out=s_dram[:, :].rearrange("(t p) one -> p (t one)", p=128),
                in_=s_all[:, :])
            nc.gpsimd.dma_start(out=s_free[0:1, :], in_=s_dram[:, :].rearrange("s one -> one s"))

            # s broadcast to 128 partitions; fold into cos/sin tables
            with tc.tile_pool(name="ps_b", bufs=2, space="PSUM") as pb:
                for w in range(W512):
                    psb = pb.tile([128, 512], fp32, tag="psb", name="psb")
                    nc.tensor.matmul(psb[:, :], ones1[0:1, :], s_free[0:1, 512 * w : 512 * (w + 1)],
                                     start=True, stop=True)
                    nc.scalar.copy(out=sbc[:, 512 * w : 512 * (w + 1)], in_=psb[:, :])
            nc.vector.tensor_mul(cosT[:, :], cosT[:, :], sbc[:, :])
            nc.vector.tensor_mul(sinT[:, :], sinT[:, :], sbc[:, :])

            # ---- phase P: projections (feature-major Q^T/K^T, token-major V) ----
            with (
                tc.tile_pool(name="ps_q", bufs=4, space="PSUM") as pq,
                tc.tile_pool(name="ps_k", bufs=2, space="PSUM") as pk,
                tc.tile_pool(name="ps_v", bufs=2, space="PSUM") as pv_,
                tc.tile_pool(name="wld", bufs=3) as wld,
                tc.tile_pool(name="rtmp", bufs=3) as rtmp,
            ):
                for w in range(W512):
                    qs = [pq.tile([128, 512], fp32, tag="psq", name="psq") for _ in range(4)]
                    ks = pk.tile([128, 512], fp32, tag="psk", name="psk")
                    vs = pv_.tile([128, 512], fp32, tag="psv", name="psv")
                    for dc in range(NDC):
                        xt_c = wld.tile([128, 512], fp32, tag="xtc", name="xtc")
                        nc.gpsimd.dma_start(out=xt_c[:, :],
                                          in_=xT[dc * 128 : (dc + 1) * 128, 512 * w : 512 * (w + 1)])
                        wq_c = wld.tile([128, CQ], fp32, tag="wqc", name="wqc")
                        nc.gpsimd.dma_start(out=wq_c[:, :], in_=wq[dc * 128 : (dc + 1) * 128, :])
                        wk_c = wld.tile([128, 128], fp32, tag="wkc", name="wkc")
                        nc.gpsimd.dma_start(out=wk_c[:, :], in_=wk[dc * 128 : (dc + 1) * 128, :])
                        wv_c = wld.tile([128, 128], fp32, tag="wvc", name="wvc")
                        nc.gpsimd.dma_start(out=wv_c[:, :], in_=wv[dc * 128 : (dc + 1) * 128, :])
                        st_, sp_ = (dc == 0), (dc == NDC - 1)
                        for ct in range(4):
                            nc.tensor.matmul(qs[ct][:, :], wq_c[:, ct * 128 : (ct + 1) * 128],
                                             xt_c[:, :], start=st_, stop=sp_)
                        nc.tensor.matmul(ks[:, :], wk_c[:, :], xt_c[:, :], start=st_, stop=sp_)
                        for vt in range(4):
                            nc.tensor.matmul(vs[:, 128 * vt : 128 * (vt + 1)],
                                             xt_c[:, 128 * vt : 128 * (vt + 1)], wv_c[:, :],
                                             start=st_, stop=sp_)
                    # RoPE + per-token scale (folded into cosT/sinT) -> SBUF
                    wsl = slice(512 * w, 512 * (w + 1))
                    for ct in range(5):
                        src = ks if ct == 4 else qs[ct]
                        dst = KT if ct == 4 else QT[ct]
                        tmp = rtmp.tile([128, 512], fp32, tag="rt", name="rt")
                        for a, bidx in ((0, 1), (1, 0), (2, 3), (3, 2)):
                            nc.vector.tensor_mul(tmp[32 * a : 32 * (a + 1), :],
                                                 src[32 * bidx : 32 * (bidx + 1), :],
                                                 sinT[32 * a : 32 * (a + 1), wsl])
                        nc.vector.tensor_mul(dst[:, wsl], src[:, :], cosT[:, wsl])
                        nc.vector.tensor_add(dst[:, wsl], dst[:, wsl], tmp[:, :])
                    for vt in range(4):
                        tt = 4 * w + vt
                        for h in range(2):
                            nc.vector.tensor_scalar_mul(
                                v_all[:, 130 * tt + 65 * h : 130 * tt + 65 * h + 64],
                                vs[:, 128 * vt + 64 * h : 128 * vt + 64 * (h + 1)],
                                s_all[:, tt : tt + 1])

            # ---- phase A: attention, transposed layout ----
            with (
                tc.tile_pool(name="ps_s", bufs=3, space="PSUM") as psc,
                tc.tile_pool(name="ps_pv", bufs=3, space="PSUM") as ppv,
                tc.tile_pool(name="ps_bc", bufs=2, space="PSUM") as pbc,
                tc.tile_pool(name="aex", bufs=4) as aex,
                tc.tile_pool(name="asm", bufs=4) as asm,
            ):
                for w in range(W512):
                    kt_max = 4 * (w + 1) if causal else NT
                    wsl = slice(512 * w, 512 * (w + 1))
                    for p in range(4):
                        pvs = [ppv.tile([65, 512], fp32, tag="pv", name="pv") for _ in range(2)]
                        for kt in range(kt_max):
                            dd = 128 * kt - 512 * w
                            for h in range(2):
                                sc = psc.tile([128, 512], fp32, tag="sc", name="sc")
                                nc.tensor.matmul(
                                    sc[:, :],
                                    KT[64 * h : 64 * (h + 1), kt * 128 : (kt + 1) * 128],
                                    QT[p][64 * h : 64 * (h + 1), wsl],
                                    start=True, stop=True)
                                ex = aex.tile([128, 512], fp32, tag="ex", name="ex")
                                nc.scalar.activation(out=ex[:, :], in_=sc[:, :], func=AF.Exp)
                                if causal and 0 <= dd <= 384:
                                    off = 384 - dd
                                    nc.vector.tensor_mul(ex[:, :], ex[:, :],
                                                         maskb[:, off : off + 512])
                                nc.tensor.matmul(
                                    pvs[h][:, :],
                                    v_all[:, 130 * kt + 65 * h : 130 * kt + 65 * (h + 1)],
                                    ex[:, :],
                                    start=(kt == 0), stop=(kt == kt_max - 1))
                        for h in range(2):
                            inv = asm.tile([1, 512], fp32, tag="inv", name="inv")
                            nc.vector.reciprocal(out=inv[:, :], in_=pvs[h][64:65, :])
                            bcp = pbc.tile([64, 512], fp32, tag="bcp", name="bcp")
                            nc.tensor.matmul(bcp[:, :], ones1[0:1, 0:64], inv[0:1, :],
                                             start=True, stop=True)
                            bc = asm.tile([64, 512], fp32, tag="bc", name="bc")
                            nc.scalar.copy(out=bc[:, :], in_=bcp[:, :])
                            nc.vector.tensor_mul(AT[p][64 * h : 64 * (h + 1), wsl],
                                                 pvs[h][0:64, :], bc[:, :])

            # ---- phase O: o_proj + residual ----
            with (
                tc.tile_pool(name="ps_o", bufs=4, space="PSUM") as po,
                tc.tile_pool(name="wo_p", bufs=5) as wop,
                tc.tile_pool(name="oep", bufs=3) as oep,
            ):
                for dw in range(4):
                    dsl = slice(512 * dw, 512 * (dw + 1))
                    wos = [wop.tile([128, 512], fp32, tag="woc", name="woc") for _ in range(4)]
                    for c in range(4):
                        nc.gpsimd.dma_start(out=wos[c][:, :], in_=wo[c * 128 : (c + 1) * 128, dsl])
                    for tt in range(NT):
                        pso = po.tile([128, 512], fp32, tag="pso", name="pso")
                        for c in range(4):
                            nc.tensor.matmul(pso[:, :], AT[c][:, tt * 128 : (tt + 1) * 128],
                                             wos[c][:, :], start=(c == 0), stop=(c == 3))
                        x_s = oep.tile([128, 512], fp32, tag="xs2", name="xs2")
                        nc.gpsimd.dma_start(out=x_s[:, :], in_=x[tt * 128 : (tt + 1) * 128, dsl])
                        xn = oep.tile([128, 512], fp32, tag="xn", name="xn")
                        nc.vector.tensor_mul(xn[:, :], x_s[:, :], rmsf[:, dsl])
                        nc.vector.tensor_scalar_mul(xn[:, :], xn[:, :], s_all[:, tt : tt + 1])
                        ob = oep.tile([128, 512], fp32, tag="ob", name="ob")
                        nc.vector.tensor_add(ob[:, :], xn[:, :], pso[:, :])
                        nc.gpsimd.dma_start(out=out[tt * 128 : (tt + 1) * 128, dsl], in_=ob[:, :])
    return nc


def _host_prep(x, rms_w, Wq, Wk, Wv, Wo):
    f32 = np.float32
    x = np.asarray(x, f32)
    rms_w = np.asarray(rms_w, f32)
    wq_full = (np.asarray(Wq, f32) * rms_w[:, None] / math.sqrt(HD)).astype(f32)
    wk_full = (np.asarray(Wk, f32) * rms_w[:, None]).astype(f32)
    wv_full = (np.asarray(Wv, f32) * rms_w[:, None]).astype(f32)
    Wo = np.asarray(Wo, f32)

    inv_f = (1.0 / (10000.0 ** (np.arange(0, HD, 2, dtype=f32) / HD))).astype(f32)
    freqs = np.arange(S, dtype=f32)[:, None] * inv_f[None, :]   # [S, 32]
    cos = np.cos(freqs).astype(f32).T                           # [32, S]
    sin = np.sin(freqs).astype(f32).T
    cosT = np.tile(np.concatenate([cos, cos], 0), (2, 1))       # [128, S]
    sinT = np.tile(np.concatenate([-sin, sin], 0), (2, 1))

    kk = np.arange(128)[:, None]
    jj = np.arange(896)[None, :]
    maskb = (jj >= kk + 384).astype(f32)

    per_core = []
    for c in range(NC):
        b, g = c // 4, c % 4
        heads = [8 * g + h for h in PERM]
        wq_g = np.ascontiguousarray(
            np.concatenate([wq_full[:, 64 * h : 64 * (h + 1)] for h in heads], axis=1))
        wo_g = np.ascontiguousarray(
            np.concatenate([Wo[64 * h : 64 * (h + 1), :] for h in heads], axis=0))
        wk_g = np.ascontiguousarray(wk_full[:, 128 * g : 128 * (g + 1)])
        wv_g = np.ascontiguousarray(wv_full[:, 128 * g : 128 * (g + 1)])
        rmsf = np.tile(rms_w, (128, 1)) if g == 0 else np.zeros((128, D), f32)
        per_core.append({
            "x": np.ascontiguousarray(x[b]),
            "xT": np.ascontiguousarray(x[b].T),
            "wq": wq_g, "wk": wk_g, "wv": wv_g, "wo": wo_g,
            "cosT": np.ascontiguousarray(cosT), "sinT": np.ascontiguousarray(sinT),
            "maskb": maskb, "rmsf": np.ascontiguousarray(rmsf.astype(f32)),
        })
    return per_core


def kernel(x, rms_w, Wq, Wk, Wv, Wo, apply_causal_mask, _trace=False):
    from concourse import bass_utils
    _install_patch()
    causal = bool(int(np.asarray(apply_causal_mask)))
    if causal not in _cache:
        _cache[causal] = _build(causal)
    nc = _cache[causal]
    in_maps = _host_prep(x, rms_w, Wq, Wk, Wv, Wo)
    r = bass_utils.run_bass_kernel_spmd(nc, in_maps, core_ids=list(range(NC)),
                                        trace=_trace)
    outs = [r.results[c]["out"] for c in range(NC)]
    full = np.stack([outs[4 * b] + outs[4 * b + 1] + outs[4 * b + 2] + outs[4 * b + 3]
                     for b in range(B)]).astype(np.float32)
    if _trace:
        kernel.last_exec_time_ns = r.exec_time_ns
    return full



# revision 4
# speedup vs baseline: 2.2139x; 2.2139x over previous
"""GQA dense-transformer block (RMSNorm + QKV + RoPE + causal GQA attention
+ o_proj + residual) on 8 trn2 NeuronCores.

Sharding: 2 (batch) x 4 (head-group tensor parallel). Core c = 4*b + g handles
batch b, q-heads 8g..8g+7, kv-heads 2g..2g+1. Each core produces a partial
o_proj output (+ the RMS-normed residual, supplied pre-multiplied by rms_w on
g==0 cores and as zeros elsewhere); the host sums the 4 partials per batch.

On-device layout: feature-major attention with bf16 matmul operands (PE runs
bf16 at 1 cycle/row vs fp32's 4). Projections compute Q^T/K^T ([head_dim,
tokens]); scores come out transposed ([k, q]) so softmax needs no transpose:
exp has no max-subtraction (scores are O(5), safe), the denominator comes from
a ones-column appended to V (PV matmul M=65), and the division is a per-head
broadcast multiply after PV. Per-token rsqrt(mean(x^2)+eps) is computed with a
ones-column matmul over squared x^T chunks (so x is only streamed once, in
bf16), then folded into the RoPE tables (Q,K) and V.

PSUM accumulate semantics: a matmul with start=True zeroes the WHOLE 2KB bank,
so when several column-slices of one bank accumulate independently (the V
subtiles), only the very first matmul in the bank may carry start=True.
"""

import math
import numpy as np

# model dims (hardcoded per contract)
B, S, D = 2, 2048, 2048
HQ, HKV, HD = 32, 8, 64
NC = 8
NG = 4            # head groups
QH = 8            # q heads per core
KH = 2            # kv heads per core
CQ = QH * HD      # 512 q cols per core
W512 = S // 512   # 4 token windows
NT = S // 128     # 16 token tiles
NDC = D // 128    # 16 contraction chunks
PERM = [0, 4, 1, 5, 2, 6, 3, 7]  # local head order: ptile p = (h=p | h=p+4)

_cache = {}
_patched = [False]


def _legalize_bir_bytes(bir):
    """Walrus in this container accepts at most ONE embedded sem-wait per TPB
    instruction ("Too many sync wait commands"). Tile emits several when an
    instruction depends on multiple DMA queues. Split the extras into
    standalone EventSemaphore (pure-wait) instructions on the same engine
    immediately before the instruction — identical blocking semantics."""
    import json
    d = json.loads(bir if isinstance(bir, str) else bir.decode())
    n_split = 0
    stack = [d]
    while stack:
        o = stack.pop()
        if isinstance(o, dict):
            insts = o.get("instructions")
            if isinstance(insts, list) and insts and isinstance(insts[0], dict) \
               and "opcode" in insts[0]:
                new = []
                for inst in insts:
                    si = inst.get("sync_info") or {}
                    ws = si.get("on_wait") or []
                    if len(ws) > 1 and isinstance(inst.get("opcode"), str) \
                       and inst.get("opcode") not in (
                            "EventSemaphore", "UnconditionalBranch",
                            "Call", "ISA"):
                        for k, w in enumerate(ws[:-1]):
                            n_split += 1
                            new.append({
                                "debug": inst.get("debug", 0),
                                "engine": inst["engine"],
                                "ins": [], "outs": [],
                                "name": f"lw{n_split}_{inst['name']}",
                                "opcode": "EventSemaphore",
                                "sync_info": {"on_update": [], "on_wait": [w]},
                            })
                        si["on_wait"] = [ws[-1]]
                    new.append(inst)
                o["instructions"] = new
            else:
                stack.extend(o.values())
        elif isinstance(o, list):
            stack.extend(o)
    return json.dumps(d).encode()


def _install_patch():
    if _patched[0]:
        return
    from concourse import bass_utils as bu
    from concourse import bass2jax as b2j
    orig = bu.compile_bir_kernel

    def patched(bir, *a, **k):
        return orig(_legalize_bir_bytes(bir), *a, **k)

    bu.compile_bir_kernel = patched
    b2j.compile_bir_kernel = patched
    _patched[0] = True


def _build(causal: bool):
    import concourse.bass as bass
    import concourse.mybir as mybir
    from concourse.tile import TileContext

    fp32 = mybir.dt.float32
    bf16 = mybir.dt.bfloat16
    AF = mybir.ActivationFunctionType

    nc = bass.Bass("TRN2")
    # host-prepped inputs (see _host_prep for layouts)
    xt = nc.dram_tensor("xt", [128, W512, NDC, 512], bf16, kind="ExternalInput")
    xr = nc.dram_tensor("xr", [S, D], fp32, kind="ExternalInput")
    wq = nc.dram_tensor("wq", [128, NDC, CQ], bf16, kind="ExternalInput")
    wk = nc.dram_tensor("wk", [128, NDC, KH * HD], bf16, kind="ExternalInput")
    wv = nc.dram_tensor("wv", [128, NDC, KH * HD], bf16, kind="ExternalInput")
    wo = nc.dram_tensor("wo", [128, 4, D], bf16, kind="ExternalInput")
    cosT_d = nc.dram_tensor("cosT", [128, S], fp32, kind="ExternalInput")
    sinT_d = nc.dram_tensor("sinT", [128, S], fp32, kind="ExternalInput")
    maskb_d = nc.dram_tensor("maskb", [128, 896], bf16, kind="ExternalInput")
    out = nc.dram_tensor("out", [S, D], fp32, kind="ExternalOutput")

    with TileContext(nc) as tc:
        with (
            tc.tile_pool(name="res", bufs=1) as res,
            tc.tile_pool(name="dram", bufs=1, space="DRAM") as dpool,
        ):
            # resident tiles
            QT = [res.tile([128, S], bf16, tag=f"qt{p}", name=f"qt{p}") for p in range(4)]
            KT = res.tile([128, S], bf16, tag="kt", name="kt")
            AT = [res.tile([128, S], bf16, tag=f"at{p}", name=f"at{p}") for p in range(4)]
            v_all = res.tile([128, NT * 130], bf16, tag="vall", name="vall")
            cosT = res.tile([128, S], fp32, tag="cosT")
            sinT = res.tile([128, S], fp32, tag="sinT")
            maskb = res.tile([128, 896], bf16, tag="maskb", name="maskb")
            wq_sb = res.tile([128, NDC, CQ], bf16, tag="wqsb", name="wq_sb")
            wk_sb = res.tile([128, NDC, KH * HD], bf16, tag="wksb", name="wk_sb")
            wv_sb = res.tile([128, NDC, KH * HD], bf16, tag="wvsb", name="wv_sb")
            wo_sb = res.tile([128, 4, D], bf16, tag="wosb", name="wo_sb")
            s_all = res.tile([128, NT], fp32, tag="sall", name="s_all")
            ones_col = res.tile([128, 1], bf16, tag="onesc", name="ones_col")
            ones1f = res.tile([1, 128], fp32, tag="ones1f", name="ones1f")
            ones64b = res.tile([1, 64], bf16, tag="ones64b", name="ones64b")
            epst = res.tile([1, 1], fp32, tag="epst", name="epst")
            s_dram = dpool.tile([S, 1], fp32, tag="sdram", name="s_dram")

            nc.vector.memset(ones_col[:, :], 1.0)
            nc.vector.memset(ones1f[:, :], 1.0)
            nc.vector.memset(ones64b[:, :], 1.0)
            nc.vector.memset(epst[:, :], float(np.finfo(np.float32).eps))
            for tt in range(NT):
                nc.vector.memset(v_all[:, 130 * tt + 64 : 130 * tt + 65], 1.0)
                nc.vector.memset(v_all[:, 130 * tt + 129 : 130 * tt + 130], 1.0)
            nc.gpsimd.dma_start(out=cosT[:, :], in_=cosT_d[:, :])
            nc.gpsimd.dma_start(out=sinT[:, :], in_=sinT_d[:, :])
            nc.gpsimd.dma_start(out=maskb[:, :], in_=maskb_d[:, :])
            nc.gpsimd.dma_start(out=wq_sb[:, :, :], in_=wq[:, :, :])
            nc.gpsimd.dma_start(out=wk_sb[:, :, :], in_=wk[:, :, :])
            nc.gpsimd.dma_start(out=wv_sb[:, :, :], in_=wv[:, :, :])
            nc.gpsimd.dma_start(out=wo_sb[:, :, :], in_=wo[:, :, :])

            # ---- phase P: stats + projections, one pass over x^T ----
            with (
                tc.tile_pool(name="ps_acc", bufs=4, space="PSUM") as pacc,
                tc.tile_pool(name="ps_v", bufs=2, space="PSUM") as pvv,
                tc.tile_pool(name="xw_p", bufs=2) as xwp,
                tc.tile_pool(name="sq_p", bufs=3) as sqp,
                tc.tile_pool(name="sw_p", bufs=2) as swp,
                tc.tile_pool(name="cf_p", bufs=2) as cfp,
                tc.tile_pool(name="rt_p", bufs=2) as rtp,
            ):
                for w in range(W512):
                    wsl = slice(512 * w, 512 * (w + 1))
                    xw = xwp.tile([128, NDC, 512], bf16, tag="xw", name="xw")
                    nc.gpsimd.dma_start(out=xw[:, :, :], in_=xt[:, w, :, :])

                    # per-token 1/sqrt(mean(x^2)+eps) via ones-column matmul
                    ps_st = pacc.tile([1, 512], fp32, tag="acc", name="ps_st")
                    for dc in range(NDC):
                        sq = sqp.tile([128, 512], bf16, tag="sq", name="sq")
                        nc.vector.tensor_mul(sq[:, :], xw[:, dc, :], xw[:, dc, :])
                        nc.tensor.matmul(ps_st[:, :], ones_col[:, :], sq[:, :],
                                         start=(dc == 0), stop=(dc == NDC - 1))
                    sq_s = swp.tile([1, 512], fp32, tag="sqs", name="sq_s")
                    nc.scalar.activation(out=sq_s[:, :], in_=ps_st[:, :], func=AF.Sqrt,
                                         bias=epst[:, 0:1], scale=1.0 / D)
                    s_w = swp.tile([1, 512], fp32, tag="sw", name="s_w")
                    nc.vector.reciprocal(out=s_w[:, :], in_=sq_s[:, :])
                    # partition-layout copy of s for V scaling + residual
                    nc.gpsimd.dma_start(
                        out=s_dram[wsl, :].rearrange("s one -> one s"),
                        in_=s_w[0:1, :])
                    nc.gpsimd.dma_start(
                        out=s_all[:, 4 * w : 4 * w + 4],
                        in_=s_dram[wsl, :].rearrange("(t p) one -> p (t one)", p=128))
                    # broadcast s to 128 partitions; fold into rope tables
                    psb = pacc.tile([128, 512], fp32, tag="acc", name="psb")
                    nc.tensor.matmul(psb[:, :], ones1f[0:1, :], s_w[0:1, :],
                                     start=True, stop=True)
                    cosF = cfp.tile([128, 512], fp32, tag="cosF", name="cosF")
                    sinF = cfp.tile([128, 512], fp32, tag="sinF", name="sinF")
                    nc.vector.tensor_mul(cosF[:, :], cosT[:, wsl], psb[:, :])
                    nc.vector.tensor_mul(sinF[:, :], sinT[:, wsl], psb[:, :])

                    # Q (4 ptiles) + K projections, feature-major, with RoPE
                    for ct in range(5):
                        ps = pacc.tile([128, 512], fp32, tag="acc", name="ps_qk")
                        for dc in range(NDC):
                            lhs = (wq_sb[:, dc, 128 * ct : 128 * (ct + 1)] if ct < 4
                                   else wk_sb[:, dc, :])
                            nc.tensor.matmul(ps[:, :], lhs, xw[:, dc, :],
                                             start=(dc == 0), stop=(dc == NDC - 1))
                        dst = KT if ct == 4 else QT[ct]
                        tmp = rtp.tile([128, 512], fp32, tag="rt", name="rt")
                        for a, bidx in ((0, 1), (1, 0), (2, 3), (3, 2)):
                            nc.vector.tensor_mul(tmp[32 * a : 32 * (a + 1), :],
                                                 ps[32 * bidx : 32 * (bidx + 1), :],
                                                 sinF[32 * a : 32 * (a + 1), :])
                        nc.vector.tensor_mul(dst[:, wsl], ps[:, :], cosF[:, :])
                        nc.vector.tensor_add(dst[:, wsl], dst[:, wsl], tmp[:, :])

                    # V projection, token-major. One PSUM bank accumulates all 4
                    # token-subtiles: only the first matmul may set start=True
                    # (start zeroes the whole bank).
                    vs = pvv.tile([128, 512], fp32, tag="psv", name="psv")
                    for vt in range(4):
                        for dc in range(NDC):
                            nc.tensor.matmul(
                                vs[:, 128 * vt : 128 * (vt + 1)],
                                xw[:, dc, 128 * vt : 128 * (vt + 1)],
                                wv_sb[:, dc, :],
                                start=(vt == 0 and dc == 0),
                                stop=(dc == NDC - 1),
                                skip_group_check=True)
                        tt = 4 * w + vt
                        for h in range(KH):
                            nc.vector.tensor_scalar_mul(
                                v_all[:, 130 * tt + 65 * h : 130 * tt + 65 * h + 64],
                                vs[:, 128 * vt + 64 * h : 128 * vt + 64 * (h + 1)],
                                s_all[:, tt : tt + 1])

            # ---- phase A: attention, transposed layout ----
            with (
                tc.tile_pool(name="ps_s", bufs=2, space="PSUM") as psc,
                tc.tile_pool(name="ps_pv", bufs=3, space="PSUM") as ppv,
                tc.tile_pool(name="ps_bc", bufs=1, space="PSUM") as pbc,
                tc.tile_pool(name="aex", bufs=4) as aex,
                tc.tile_pool(name="asm", bufs=4) as asm,
            ):
                for w in range(W512):
                    kt_max = 4 * (w + 1) if causal else NT
                    wsl = slice(512 * w, 512 * (w + 1))
                    for p in range(4):
                        pvs = [ppv.tile([65, 512], fp32, tag="pv", name="pv")
                               for _ in range(2)]
                        for kt in range(kt_max):
                            dd = 128 * kt - 512 * w
                            sc = psc.tile([128, 1024], fp32, tag="sc", name="sc")
                            for h in range(2):
                                nc.tensor.matmul(
                                    sc[:, 512 * h : 512 * (h + 1)],
                                    KT[64 * h : 64 * (h + 1), kt * 128 : (kt + 1) * 128],
                                    QT[p][64 * h : 64 * (h + 1), wsl],
                                    start=True, stop=True)
                            ex = aex.tile([128, 1024], bf16, tag="ex", name="ex")
                            nc.scalar.activation(out=ex[:, :], in_=sc[:, :], func=AF.Exp)
                            if causal and 0 <= dd <= 384:
                                off = 384 - dd
                                for h in range(2):
                                    nc.vector.tensor_mul(
                                        ex[:, 512 * h : 512 * (h + 1)],
                                        ex[:, 512 * h : 512 * (h + 1)],
                                        maskb[:, off : off + 512])
                            for h in range(2):
                                nc.tensor.matmul(
                                    pvs[h][:, :],
                                    v_all[:, 130 * kt + 65 * h : 130 * kt + 65 * (h + 1)],
                                    ex[:, 512 * h : 512 * (h + 1)],
                                    start=(kt == 0), stop=(kt == kt_max - 1))
                        for h in range(2):
                            inv = asm.tile([1, 512], bf16, tag="inv", name="inv")
                            with nc.allow_low_precision("softmax denom bcast in bf16"):
                                nc.vector.reciprocal(out=inv[:, :], in_=pvs[h][64:65, :])
                            bcp = pbc.tile([64, 512], fp32, tag="bcp", name="bcp")
                            nc.tensor.matmul(bcp[:, :], ones64b[0:1, :], inv[0:1, :],
                                             start=True, stop=True)
                            bc = asm.tile([64, 512], bf16, tag="bc", name="bc")
                            nc.vector.tensor_scalar_mul(bc[:, :], bcp[:, :], 1.0)
                            nc.vector.tensor_mul(AT[p][64 * h : 64 * (h + 1), wsl],
                                                 pvs[h][0:64, :], bc[:, :])

            # ---- phase O: o_proj + scaled residual ----
            with (
                tc.tile_pool(name="ps_o", bufs=4, space="PSUM") as po,
                tc.tile_pool(name="oxp", bufs=3) as oxp,
                tc.tile_pool(name="oep", bufs=4) as oep,
            ):
                for dw in range(4):
                    dsl = slice(512 * dw, 512 * (dw + 1))
                    for tt in range(NT):
                        pso = po.tile([128, 512], fp32, tag="pso", name="pso")
                        for c in range(4):
                            nc.tensor.matmul(pso[:, :],
                                             AT[c][:, tt * 128 : (tt + 1) * 128],
                                             wo_sb[:, c, dsl],
                                             start=(c == 0), stop=(c == 3))
                        x_s = oxp.tile([128, 512], fp32, tag="xs2", name="xs2")
                        nc.gpsimd.dma_start(out=x_s[:, :],
                                            in_=xr[tt * 128 : (tt + 1) * 128, dsl])
                        xn = oep.tile([128, 512], fp32, tag="xn", name="xn")
                        nc.scalar.activation(out=xn[:, :], in_=x_s[:, :], func=AF.Copy,
                                             scale=s_all[:, tt : tt + 1])
                        ob = oep.tile([128, 512], fp32, tag="ob", name="ob")
                        nc.vector.tensor_add(ob[:, :], xn[:, :], pso[:, :])
                        nc.sync.dma_start(out=out[tt * 128 : (tt + 1) * 128, dsl],
                                          in_=ob[:, :])
    return nc


def _host_prep(x, rms_w, Wq, Wk, Wv, Wo):
    import ml_dtypes
    f32 = np.float32
    bf = ml_dtypes.bfloat16
    x = np.asarray(x, f32)
    rms_w = np.asarray(rms_w, f32)
    wq_full = (np.asarray(Wq, f32) * rms_w[:, None] / math.sqrt(HD)).astype(f32)
    wk_full = (np.asarray(Wk, f32) * rms_w[:, None]).astype(f32)
    wv_full = (np.asarray(Wv, f32) * rms_w[:, None]).astype(f32)
    Wo = np.asarray(Wo, f32)

    inv_f = (1.0 / (10000.0 ** (np.arange(0, HD, 2, dtype=f32) / HD))).astype(f32)
    freqs = np.arange(S, dtype=f32)[:, None] * inv_f[None, :]   # [S, 32]
    cos = np.cos(freqs).astype(f32).T                           # [32, S]
    sin = np.sin(freqs).astype(f32).T
    cosT = np.tile(np.concatenate([cos, cos], 0), (2, 1))       # [128, S]
    sinT = np.tile(np.concatenate([-sin, sin], 0), (2, 1))

    kk = np.arange(128)[:, None]
    jj = np.arange(896)[None, :]
    maskb = (jj >= kk + 384).astype(bf)

    def chunked(wfull):
        # [D, C] -> [128, NDC, C]: row 128*dc+p goes to [p, dc, :]
        C = wfull.shape[1]
        return np.ascontiguousarray(
            wfull.reshape(NDC, 128, C).transpose(1, 0, 2)).astype(bf)

    zeros_xr = np.zeros((S, D), f32)
    per_core = []
    for c in range(NC):
        b, g = c // 4, c % 4
        heads = [8 * g + h for h in PERM]
        wq_g = np.concatenate([wq_full[:, 64 * h : 64 * (h + 1)] for h in heads], axis=1)
        wo_g = np.concatenate([Wo[64 * h : 64 * (h + 1), :] for h in heads], axis=0)
        wk_g = wk_full[:, 128 * g : 128 * (g + 1)]
        wv_g = wv_full[:, 128 * g : 128 * (g + 1)]
        xT = x[b].T.astype(bf)                                   # [D, S]
        # [128, W512, NDC, 512]: xt[p, w, dc, s] = xT[128*dc+p, 512*w+s]
        xt_r = np.ascontiguousarray(
            xT.reshape(NDC, 128, W512, 512).transpose(1, 2, 0, 3))
        wo_r = np.ascontiguousarray(
            wo_g.reshape(4, 128, D).transpose(1, 0, 2)).astype(bf)  # [128, 4, D]
        xr_g = np.ascontiguousarray(x[b] * rms_w[None, :]) if g == 0 else zeros_xr
        per_core.append({
            "xt": xt_r, "xr": xr_g,
            "wq": chunked(wq_g), "wk": chunked(wk_g), "wv": chunked(wv_g),
            "wo": wo_r,
            "cosT": np.ascontiguousarray(cosT), "sinT": np.ascontiguousarray(sinT),
            "maskb": maskb,
        })
    return per_core


def kernel(x, rms_w, Wq, Wk, Wv, Wo, apply_causal_mask, _trace=False):
    from concourse import bass_utils
    _install_patch()
    causal = bool(int(np.asarray(apply_causal_mask)))
    if causal not in _cache:
        _cache[causal] = _build(causal)
    nc = _cache[causal]
    in_maps = _host_prep(x, rms_w, Wq, Wk, Wv, Wo)
    r = bass_utils.run_bass_kernel_spmd(nc, in_maps, core_ids=list(range(NC)),
                                        trace=_trace)
    outs = [r.results[c]["out"] for c in range(NC)]
    full = np.stack([outs[4 * b] + outs[4 * b + 1] + outs[4 * b + 2] + outs[4 * b + 3]
                     for b in range(B)]).astype(np.float32)
    if _trace:
        kernel.last_exec_time_ns = r.exec_time_ns
    return full
